# revision 18
# baseline (speedup 1.0000x reference)
"""Two-layer GAT (PyG GATConv x2 + linear head) on 8 TRN2 NeuronCores via Bass/Tile.

Self-contained kernel: kernel(**inputs) takes the FULL inputs of
reference.setup_inputs() and returns the FULL [100000, 1] float32 output.

Strategy (degree-sorted dst sharding, fully batched):
- Host sorts nodes by in-degree and deals 128-node blocks round-robin to the
  8 cores; every block's edge slots form a dense [128, Tb] grid (Tb = max
  degree in the block's octet) so there is NO overflow path at all.
- Node tables [h | al_src] are built per-core with matmuls and AllGathered to
  a replicated DRAM table; a sentinel row (al_src = -1e30) makes padded edge
  slots contribute exactly zero to the segment softmax.
- Per-edge work is batched over groups of blocks: one multi-row indirect DMA
  gathers thousands of table rows per instruction, and attention
  (leaky-relu, exp, weighting, segment sums) runs as a handful of 4D-AP DVE
  ops per group instead of per-128-edge-tile instructions.
"""
import sys
import types
import numpy as np
import concourse.bass as bass
import concourse.mybir as mybir
import concourse.tile as tile
from concourse import bacc
from concourse.bass_utils import run_bass_kernel_spmd


def _install_axon_ntff_shim():
    """Allow trace=True under axon when antenv.axon_hooks is absent."""
    if "antenv.axon_hooks" in sys.modules:
        return True
    try:
        from trn_agent_boot.trn_boot import _ntff_profile_via_ctypes
        hook = _ntff_profile_via_ctypes("/opt/axon/libaxon_pjrt.so")
        mod = types.ModuleType("antenv.axon_hooks")
        mod._hook = hook
        mod.set_axon_ntff_profile_hook = lambda h: setattr(mod, "_hook", h)
        mod.get_axon_ntff_profile_hook = lambda: mod._hook
        sys.modules["antenv.axon_hooks"] = mod
        import antenv
        antenv.axon_hooks = mod
        return True
    except Exception:
        return False


F32 = mybir.dt.float32
I32 = mybir.dt.int32
AX = mybir.AxisListType.X
NEG_SLOPE = 0.2
EPS = 1e-16

N_NODES = 100000
F_IN = 512
SHARD = 12544          # 98 blocks of 128 per core; 8*12544 = 100352
NB = 98
NPAD = 8 * SHARD
SLOTCAP = 448          # max edge-slot columns per group
GCHUNK = 64            # offset columns per indirect DMA (8192 descriptors)


class Cfg:
    def __init__(self, groups):
        # groups: list of (lb0, G, TG); consecutive blocks, same for all cores
        self.groups = groups
        self.NT = sum(G * TG for (_, G, TG) in groups)
        self.SLOTMAX = max(G * TG for (_, G, TG) in groups)
        self.GMAX = max(G for (_, G, TG) in groups)
        assert self.SLOTMAX <= 640, self.SLOTMAX
        assert self.NT <= 4608, self.NT


def build(cfg: Cfg):
    nc = bacc.Bacc("TRN2", target_bir_lowering=False, debug=False, num_devices=8)

    xT = nc.dram_tensor("xT", [F_IN, SHARD], F32, kind="ExternalInput")
    pack1 = nc.dram_tensor("pack1", [F_IN, 24], F32, kind="ExternalInput")
    pk2repD = nc.dram_tensor("pk2rep", [128, 192], F32, kind="ExternalInput")
    b1rep = nc.dram_tensor("b1rep", [128, 16], F32, kind="ExternalInput")
    b2rep = nc.dram_tensor("b2rep", [128, 4], F32, kind="ExternalInput")
    wfcrep = nc.dram_tensor("wfcrep", [128, 4], F32, kind="ExternalInput")
    offsD = nc.dram_tensor("offs", [128, cfg.NT], I32, kind="ExternalInput")
    smulD = nc.dram_tensor("smul", [128, NB], F32, kind="ExternalInput")
    yout = nc.dram_tensor("yout", [128, NB], F32, kind="ExternalOutput")

    bfc_const = None  # bfc folded on host into b2? no - applied as immediate

    with tile.TileContext(nc) as tc:
        with (
            tc.tile_pool(name="const", bufs=1) as cp,
            tc.tile_pool(name="ps2", bufs=2, space="PSUM") as ps2,
            tc.tile_pool(name="sb", bufs=1) as sb,
            tc.tile_pool(name="db", bufs=2) as db,
            tc.tile_pool(name="dram", bufs=1, space="DRAM") as dp,
        ):
            SLOTMAX, GMAX, NT = cfg.SLOTMAX, cfg.GMAX, cfg.NT

            # ---- persistent SBUF tiles ----
            offs = cp.tile([128, NT], I32)
            nc.sync.dma_start(out=offs[:], in_=offsD[:, :])
            pk1 = cp.tile([128, 96], F32)
            nc.sync.dma_start(
                out=pk1[:].rearrange("p (k j) -> p k j", j=24),
                in_=pack1[:, :].rearrange("(k p) j -> p k j", p=128),
            )
            pk2r = cp.tile([128, 192], F32)
            nc.sync.dma_start(out=pk2r[:], in_=pk2repD[:, :])
            b1c = cp.tile([128, 16], F32)
            nc.sync.dma_start(out=b1c[:], in_=b1rep[:, :])
            b2c = cp.tile([128, 4], F32)
            nc.sync.dma_start(out=b2c[:], in_=b2rep[:, :])
            wfcc = cp.tile([128, 4], F32)
            nc.sync.dma_start(out=wfcc[:], in_=wfcrep[:, :])
            smul = cp.tile([128, NB], F32)
            nc.sync.dma_start(out=smul[:], in_=smulD[:, :])
            adT1 = cp.tile([128, NB * 4], F32)
            adT2 = cp.tile([128, NB * 4], F32)
            h1sb = cp.tile([128, NB * 16], F32)
            ysb = cp.tile([128, NB], F32)

            # DRAM intermediates
            g1loc = dp.tile([SHARD, 20], F32)
            tbl1 = dp.tile([NPAD + 1, 20], F32)
            g2loc = dp.tile([SHARD, 8], F32)
            tbl2 = dp.tile([NPAD + 1, 8], F32)

            # sentinel rows (padded slots gather these; al_src=-1e30 => exp=0)
            sent1 = cp.tile([1, 20], F32)
            nc.vector.memset(sent1[:], 0.0)
            nc.vector.memset(sent1[:, 16:20], -1e30)
            nc.sync.dma_start(out=tbl1[NPAD : NPAD + 1, :], in_=sent1[:])
            sent2 = cp.tile([1, 8], F32)
            nc.vector.memset(sent2[:], 0.0)
            nc.vector.memset(sent2[:, 4:8], -1e30)
            nc.sync.dma_start(out=tbl2[NPAD : NPAD + 1, :], in_=sent2[:])

            # ---- P1: layer-1 node tables  g1 = x @ [W1 | W1A1s | W1A1d] ----
            SGN = 11  # small enough to double-buffer x loads against matmuls
            SGS = [(b0, min(SGN, NB - b0)) for b0 in range(0, NB, SGN)]
            for (b0, nsg) in SGS:
                acc = ps2.tile([128, SGN * 24], F32, tag="p1acc")
                xall = db.tile([128, 4 * SGN * 128], F32, tag="xall")
                for k in range(4):
                    nc.sync.dma_start(
                        out=xall[:, k * SGN * 128 : k * SGN * 128 + nsg * 128],
                        in_=xT[k * 128 : (k + 1) * 128, b0 * 128 : (b0 + nsg) * 128],
                    )
                for i in range(nsg):
                    for k in range(4):
                        nc.tensor.matmul(
                            acc[:, i * 24 : (i + 1) * 24],
                            lhsT=xall[:, (k * SGN + i) * 128 : (k * SGN + i + 1) * 128],
                            rhs=pk1[:, k * 24 : (k + 1) * 24],
                            start=(k == 0),
                            stop=(k == 3),
                        )
                g1r = sb.tile([128, SGN * 24], F32, tag="g1r")
                nc.vector.tensor_copy(out=g1r[:, : nsg * 24], in_=acc[:, : nsg * 24])
                nc.sync.dma_start(
                    out=g1loc[b0 * 128 : (b0 + nsg) * 128, :].rearrange(
                        "(i p) j -> p i j", p=128
                    ),
                    in_=g1r[:, : nsg * 24].rearrange("p (i j) -> p i j", j=24)[
                        :, :, 0:20
                    ],
                )
                nc.vector.tensor_copy(
                    out=adT1[:, b0 * 4 : (b0 + nsg) * 4].rearrange(
                        "p (i j) -> p i j", j=4
                    ),
                    in_=g1r[:, : nsg * 24].rearrange("p (i j) -> p i j", j=24)[
                        :, :, 20:24
                    ],
                )

            # ---- P2: all-gather layer-1 table ----
            nc.gpsimd.collective_compute(
                "AllGather", mybir.AluOpType.bypass,
                replica_groups=[list(range(8))],
                ins=[g1loc.opt()], outs=[tbl1[0:NPAD, :].opt()],
            )

            # ---- P3: layer-1 edge phase, one pass per group ----
            def edge_phase(tbl, loc, nrow, adT, hcols, out_cb):
                """nrow: table row width; hcols: payload cols (h), then
                al_src at [hcols:hcols+4]. loc: core-local table (affine
                source for self-loop rows). out_cb(lb0, G, num, rec)
                consumes segment results: num [128,G*hcols], rec [128,G*4]."""
                cbase = 0
                for (lb0, G, TG) in cfg.groups:
                    cols = G * TG
                    gg = db.tile([128, SLOTMAX * 20], F32, tag="gg")
                    # one indirect DMA per 128-edge column: HW consumes one
                    # offset per partition-row and fetches a contiguous run
                    # of dest-size elements, so batching offset columns into
                    # a single DMA is not possible on this image.
                    for c in range(cols):
                        nc.gpsimd.indirect_dma_start(
                            out=gg[:, c * nrow : (c + 1) * nrow],
                            out_offset=None,
                            in_=tbl[:, :],
                            in_offset=bass.IndirectOffsetOnAxis(
                                ap=offs[:, cbase + c : cbase + c + 1], axis=0
                            ),
                        )
                    # views: (g, j, t) with j = col-in-row
                    gv = gg[:, : cols * nrow].rearrange(
                        "p (g t j) -> p g j t", t=TG, j=nrow
                    )
                    # t4 = al_src[src] + al_dst[dst]; layout (g, h, t)
                    adg = (
                        adT[:, lb0 * 4 : (lb0 + G) * 4]
                        .rearrange("p (g j) -> p g j", j=4)
                        .unsqueeze(3)
                        .broadcast_to([128, G, 4, TG])
                    )
                    t4 = sb.tile([128, SLOTMAX * 4], F32, tag="t4")
                    nc.vector.tensor_add(
                        out=t4[:, : cols * 4].rearrange(
                            "p (g j t) -> p g j t", j=4, t=TG
                        ),
                        in0=gv[:, :, hcols : hcols + 4, :],
                        in1=adg,
                    )
                    lr = sb.tile([128, SLOTMAX * 4], F32, tag="lr")
                    nc.vector.scalar_tensor_tensor(
                        out=lr[:, : cols * 4], in0=t4[:, : cols * 4],
                        scalar=NEG_SLOPE, in1=t4[:, : cols * 4],
                        op0=mybir.AluOpType.mult, op1=mybir.AluOpType.max,
                    )
                    ex = sb.tile([128, SLOTMAX * 4], F32, tag="ex")
                    nc.scalar.activation(
                        ex[:, : cols * 4], lr[:, : cols * 4],
                        mybir.ActivationFunctionType.Exp,
                    )
                    exv = ex[:, : cols * 4].rearrange(
                        "p (g h t) -> p g h t", h=4, t=TG
                    )
                    # denominator
                    den = sb.tile([128, GMAX * 4], F32, tag="den")
                    nc.vector.reduce_sum(out=den[:, : G * 4], in_=exv, axis=AX)
                    # weighted payload + numerator; CH = channels per head
                    CH = hcols // 4
                    payw = sb.tile([128, SLOTMAX * 16], F32, tag="payw")
                    if CH > 1:
                        pwv = payw[:, : cols * hcols].rearrange(
                            "p (g h c t) -> p g h c t", h=4, c=CH, t=TG
                        )
                        for h in range(4):
                            nc.vector.tensor_mul(
                                out=pwv[:, :, h : h + 1, :, :].squeeze(2),
                                in0=gv[:, :, h * CH : (h + 1) * CH, :],
                                in1=exv[:, :, h : h + 1, :].broadcast_to(
                                    [128, G, CH, TG]
                                ),
                            )
                    else:
                        nc.vector.tensor_mul(
                            out=payw[:, : cols * 4].rearrange(
                                "p (g h t) -> p g h t", h=4, t=TG
                            ),
                            in0=gv[:, :, 0:4, :],
                            in1=exv,
                        )
                    nums = sb.tile([128, GMAX * 16], F32, tag="nums")
                    nc.vector.reduce_sum(
                        out=nums[:, : G * hcols],
                        in_=payw[:, : cols * hcols].rearrange(
                            "p (q t) -> p q t", t=TG
                        ),
                        axis=AX,
                    )
                    # self-loop contribution: dst's own row, affine load,
                    # scaled by the self-edge multiplicity stream
                    selfb = sb.tile([128, GMAX * 20], F32, tag="selfb")
                    nc.sync.dma_start(
                        out=selfb[:, : G * nrow].rearrange(
                            "p (g j) -> p g j", j=nrow
                        ),
                        in_=loc[lb0 * 128 : (lb0 + G) * 128, :].rearrange(
                            "(g p) j -> p g j", p=128
                        ),
                    )
                    sv = selfb[:, : G * nrow].rearrange("p (g j) -> p g j", j=nrow)
                    t4s = sb.tile([128, GMAX * 4], F32, tag="t4s")
                    nc.vector.tensor_add(
                        out=t4s[:, : G * 4].rearrange("p (g j) -> p g j", j=4),
                        in0=sv[:, :, hcols : hcols + 4],
                        in1=adT[:, lb0 * 4 : (lb0 + G) * 4].rearrange(
                            "p (g j) -> p g j", j=4
                        ),
                    )
                    lrs = sb.tile([128, GMAX * 4], F32, tag="lrs")
                    nc.vector.scalar_tensor_tensor(
                        out=lrs[:, : G * 4], in0=t4s[:, : G * 4],
                        scalar=NEG_SLOPE, in1=t4s[:, : G * 4],
                        op0=mybir.AluOpType.mult, op1=mybir.AluOpType.max,
                    )
                    exs = sb.tile([128, GMAX * 4], F32, tag="exs")
                    nc.scalar.activation(
                        exs[:, : G * 4], lrs[:, : G * 4],
                        mybir.ActivationFunctionType.Exp,
                    )
                    nc.vector.tensor_mul(
                        out=exs[:, : G * 4].rearrange("p (g j) -> p g j", j=4),
                        in0=exs[:, : G * 4].rearrange("p (g j) -> p g j", j=4),
                        in1=smul[:, lb0 : lb0 + G]
                        .unsqueeze(2)
                        .broadcast_to([128, G, 4]),
                    )
                    nc.vector.tensor_add(
                        out=den[:, : G * 4], in0=den[:, : G * 4],
                        in1=exs[:, : G * 4],
                    )
                    pws = sb.tile([128, GMAX * 16], F32, tag="pws")
                    if hcols == 16:
                        nc.vector.tensor_mul(
                            out=pws[:, : G * 16].rearrange(
                                "p (g h c) -> p g h c", h=4, c=4
                            ),
                            in0=sv[:, :, 0:16].rearrange(
                                "p g (h c) -> p g h c", c=4
                            ),
                            in1=exs[:, : G * 4]
                            .rearrange("p (g h) -> p g h", h=4)
                            .unsqueeze(3)
                            .broadcast_to([128, G, 4, 4]),
                        )
                    else:
                        nc.vector.tensor_mul(
                            out=pws[:, : G * 4].rearrange("p (g j) -> p g j", j=4),
                            in0=sv[:, :, 0:4],
                            in1=exs[:, : G * 4].rearrange("p (g j) -> p g j", j=4),
                        )
                    nc.vector.tensor_add(
                        out=nums[:, : G * hcols], in0=nums[:, : G * hcols],
                        in1=pws[:, : G * hcols],
                    )
                    # reciprocal of denominator (+ one NR step)
                    sp = sb.tile([128, GMAX * 4], F32, tag="sp")
                    nc.vector.tensor_scalar_add(sp[:, : G * 4], den[:, : G * 4], EPS)
                    rec = sb.tile([128, GMAX * 4], F32, tag="rec")
                    nc.vector.reciprocal(rec[:, : G * 4], sp[:, : G * 4])
                    nr = sb.tile([128, GMAX * 4], F32, tag="nr")
                    nc.vector.tensor_mul(
                        out=nr[:, : G * 4], in0=sp[:, : G * 4], in1=rec[:, : G * 4]
                    )
                    nc.vector.tensor_scalar_mul(nr[:, : G * 4], nr[:, : G * 4], -1.0)
                    nc.vector.tensor_scalar_add(nr[:, : G * 4], nr[:, : G * 4], 2.0)
                    nc.vector.tensor_mul(
                        out=rec[:, : G * 4], in0=rec[:, : G * 4], in1=nr[:, : G * 4]
                    )
                    out_cb(lb0, G, nums, rec)
                    cbase += cols

            def l1_finalize(lb0, G, nums, rec):
                # h1 = elu(num*rec + b1)
                o16 = sb.tile([128, GMAX * 16], F32, tag="o16")
                nc.vector.tensor_mul(
                    out=o16[:, : G * 16].rearrange("p (g h c) -> p g h c", h=4, c=4),
                    in0=nums[:, : G * 16].rearrange("p (g h c) -> p g h c", h=4, c=4),
                    in1=rec[:, : G * 4]
                    .rearrange("p (g h) -> p g h", h=4)
                    .unsqueeze(3)
                    .broadcast_to([128, G, 4, 4]),
                )
                nc.vector.tensor_add(
                    out=o16[:, : G * 16].rearrange("p (g j) -> p g j", j=16),
                    in0=o16[:, : G * 16].rearrange("p (g j) -> p g j", j=16),
                    in1=b1c[:, :].unsqueeze(1).broadcast_to([128, G, 16]),
                )
                m0 = sb.tile([128, GMAX * 16], F32, tag="m0")
                nc.vector.tensor_scalar_min(m0[:, : G * 16], o16[:, : G * 16], 0.0)
                em = sb.tile([128, GMAX * 16], F32, tag="em")
                nc.scalar.activation(
                    em[:, : G * 16], m0[:, : G * 16],
                    mybir.ActivationFunctionType.Exp,
                )
                nc.vector.tensor_scalar_add(em[:, : G * 16], em[:, : G * 16], -1.0)
                nc.vector.tensor_tensor(
                    out=h1sb[:, lb0 * 16 : (lb0 + G) * 16],
                    in0=o16[:, : G * 16],
                    in1=em[:, : G * 16],
                    op=mybir.AluOpType.max,
                )

            edge_phase(tbl1, g1loc, 20, adT1, 16, l1_finalize)

            # ---- P4: layer-2 node tables  g2 = h1 @ [W2 | W2A2s | W2A2d] ----
            g2sb = cp.tile([128, NB * 12], F32)
            for j in range(12):
                tmp = sb.tile([128, NB * 16], F32, tag="p4tmp")
                nc.vector.tensor_mul(
                    out=tmp[:].rearrange("p (b k) -> p b k", k=16),
                    in0=h1sb[:].rearrange("p (b k) -> p b k", k=16),
                    in1=pk2r[:, j * 16 : (j + 1) * 16]
                    .unsqueeze(1)
                    .broadcast_to([128, NB, 16]),
                )
                nc.vector.reduce_sum(
                    out=g2sb[:].rearrange("p (b j) -> p b j", j=12)[:, :, j : j + 1],
                    in_=tmp[:].rearrange("p (b k) -> p b k", k=16),
                    axis=AX,
                )
            nc.vector.tensor_copy(
                out=adT2[:].rearrange("p (b j) -> p b j", j=4),
                in_=g2sb[:].rearrange("p (b j) -> p b j", j=12)[:, :, 8:12],
            )
            nc.sync.dma_start(
                out=g2loc[:, :].rearrange("(b p) j -> p b j", p=128),
                in_=g2sb[:].rearrange("p (b j) -> p b j", j=12)[:, :, 0:8],
            )

            # ---- P5: all-gather layer-2 table ----
            nc.gpsimd.collective_compute(
                "AllGather", mybir.AluOpType.bypass,
                replica_groups=[list(range(8))],
                ins=[g2loc.opt()], outs=[tbl2[0:NPAD, :].opt()],
            )

            # ---- P6: layer-2 edge phase + fc head ----
            def l2_finalize(lb0, G, nums, rec):
                o4 = sb.tile([128, GMAX * 4], F32, tag="o4")
                nc.vector.tensor_mul(
                    out=o4[:, : G * 4], in0=nums[:, : G * 4], in1=rec[:, : G * 4]
                )
                nc.vector.tensor_add(
                    out=o4[:, : G * 4].rearrange("p (g j) -> p g j", j=4),
                    in0=o4[:, : G * 4].rearrange("p (g j) -> p g j", j=4),
                    in1=b2c[:, :].unsqueeze(1).broadcast_to([128, G, 4]),
                )
                nc.vector.tensor_mul(
                    out=o4[:, : G * 4].rearrange("p (g j) -> p g j", j=4),
                    in0=o4[:, : G * 4].rearrange("p (g j) -> p g j", j=4),
                    in1=wfcc[:, :].unsqueeze(1).broadcast_to([128, G, 4]),
                )
                nc.vector.reduce_sum(
                    out=ysb[:, lb0 : lb0 + G],
                    in_=o4[:, : G * 4].rearrange("p (g j) -> p g j", j=4),
                    axis=AX,
                )

            edge_phase(tbl2, g2loc, 8, adT2, 4, l2_finalize)

            nc.sync.dma_start(out=yout[:, :], in_=ysb[:])
    nc.compile()
    return nc


def compute_groups(T_lb):
    """T_lb: [NB] per-octet tile heights (non-increasing). Returns group list."""
    groups = []
    lb = 0
    while lb < NB:
        TG = max(int(T_lb[lb]), 1)
        G = min(NB - lb, max(1, SLOTCAP // TG))
        groups.append((lb, G, TG))
        lb += G
    return groups


def host_prep(inputs: dict):
    x = np.asarray(inputs["x"], np.float32)
    ei = np.asarray(inputs["edge_index"])
    src = np.concatenate([ei[0], np.arange(N_NODES, dtype=np.int64)]).astype(np.int64)
    dst = np.concatenate([ei[1], np.arange(N_NODES, dtype=np.int64)]).astype(np.int64)

    # self edges (added loops + natural (i,i)) go through the affine path
    selfmask = src == dst
    m = np.bincount(dst[selfmask], minlength=NPAD).astype(np.float32)
    src = src[~selfmask]
    dst = dst[~selfmask]

    deg = np.bincount(dst, minlength=NPAD)
    order = np.argsort(-deg, kind="stable")           # new_id -> old_id
    inv = np.empty(NPAD, np.int64)
    inv[order] = np.arange(NPAD)
    degs = deg[order]
    morder = m[order]                                 # self multiplicity

    T_lb = degs[np.arange(NB) * 1024]                 # octet max degrees
    groups = compute_groups(T_lb)
    cfg = Cfg(groups)

    colarr = np.zeros(NB, np.int64)                   # per-block column base
    base = 0
    for (lb0, G, TG) in groups:
        for i in range(G):
            colarr[lb0 + i] = base + i * TG
        base += G * TG
    assert base == cfg.NT

    nd = inv[dst]
    ns = inv[src]
    eorder = np.argsort(nd, kind="stable")
    nd_s = nd[eorder]
    ns_s = ns[eorder]
    E = len(nd_s)
    starts = np.zeros(NPAD + 1, np.int64)
    np.cumsum(np.bincount(nd_s, minlength=NPAD), out=starts[1:])
    rank = np.arange(E, dtype=np.int64) - starts[nd_s]

    g_d = nd_s // 128
    core_s = g_d % 8
    lb_s = g_d // 8
    p_s = nd_s % 128
    gsrc = ns_s // 128
    trow = ((gsrc % 8) * SHARD + (gsrc // 8) * 128 + (ns_s % 128)).astype(np.int64)
    col = colarr[lb_s] + rank

    offs_all = np.full((8, 128, cfg.NT), NPAD, np.int32)
    offs_all[core_s, p_s, col] = trow.astype(np.int32)

    # weight packs (host precompute)
    W1 = np.asarray(inputs["W1"], np.float32)
    a_src1 = np.asarray(inputs["a_src1"], np.float32)
    a_dst1 = np.asarray(inputs["a_dst1"], np.float32)
    A1s = np.zeros((16, 4), np.float32)
    A1d = np.zeros((16, 4), np.float32)
    for h in range(4):
        A1s[h * 4 : h * 4 + 4, h] = a_src1[h]
        A1d[h * 4 : h * 4 + 4, h] = a_dst1[h]
    pack1 = np.concatenate([W1, W1 @ A1s, W1 @ A1d], axis=1)  # [512, 24]

    W2 = np.asarray(inputs["W2"], np.float32)
    a2s = np.asarray(inputs["a_src2"], np.float32)[:, 0]
    a2d = np.asarray(inputs["a_dst2"], np.float32)[:, 0]
    pack2 = np.concatenate([W2, W2 * a2s[None, :], W2 * a2d[None, :]], axis=1)  # [16,12]
    pk2rep = np.tile(pack2.T.reshape(1, 192), (128, 1)).astype(np.float32)

    b1rep = np.tile(np.asarray(inputs["b1"], np.float32)[None, :], (128, 1))
    b2rep = np.tile(np.asarray(inputs["b2"], np.float32)[None, :], (128, 1))
    wfcrep = np.tile(np.asarray(inputs["Wfc"], np.float32)[:, 0][None, :], (128, 1))
    bfc = float(np.asarray(inputs["bfc"])[0])

    xp = np.zeros((NPAD, F_IN), np.float32)
    vm = order < N_NODES
    xp[vm] = x[order[vm]]
    xpb = xp.reshape(784, 128, F_IN)

    mb = morder.reshape(784, 128)
    in_maps = []
    for c in range(8):
        xT_c = np.ascontiguousarray(
            xpb[c::8].reshape(SHARD, F_IN).T
        )
        smul_c = np.ascontiguousarray(mb[c::8].T)     # [128, NB]
        in_maps.append({
            "xT": xT_c, "pack1": pack1, "pk2rep": pk2rep,
            "b1rep": b1rep, "b2rep": b2rep, "wfcrep": wfcrep,
            "offs": np.ascontiguousarray(offs_all[c]),
            "smul": smul_c,
        })
    return cfg, in_maps, order, vm, bfc


def assemble_output(results, order, vm, bfc):
    ynew = np.zeros(NPAD, np.float32)
    yb = ynew.reshape(784, 128)
    for c in range(8):
        yb[c::8] = np.asarray(results[c]["yout"]).T  # [NB,128]
    y = np.empty(N_NODES, np.float32)
    y[order[vm]] = ynew[vm] + bfc
    return y[:, None]


LAST_EXEC_NS = None


def run(inputs: dict, trace: bool = False):
    cfg, in_maps, order, vm, bfc = host_prep(inputs)
    nc = build(cfg)
    res = run_bass_kernel_spmd(nc, in_maps, core_ids=list(range(8)), trace=trace)
    y = assemble_output(res.results, order, vm, bfc)
    return y, res


def kernel(**inputs) -> np.ndarray:
    global LAST_EXEC_NS
    trace = _install_axon_ntff_shim()
    try:
        y, res = run(inputs, trace=trace)
    except Exception:
        if not trace:
            raise
        y, res = run(inputs, trace=False)
    LAST_EXEC_NS = res.exec_time_ns
    return np.ascontiguousarray(y.astype(np.float32))


# revision 21
# speedup vs baseline: 1.0035x; 1.0035x over previous
"""Two-layer GAT (PyG GATConv x2 + linear head) on 8 TRN2 NeuronCores via Bass/Tile.

Self-contained kernel: kernel(**inputs) takes the FULL inputs of
reference.setup_inputs() and returns the FULL [100000, 1] float32 output.

Strategy (degree-sorted dst sharding, fully batched):
- Host sorts nodes by in-degree and deals 128-node blocks round-robin to the
  8 cores; every block's edge slots form a dense [128, Tb] grid (Tb = max
  degree in the block's octet) so there is NO overflow path at all.
- Node tables [h | al_src] are built per-core with matmuls and AllGathered to
  a replicated DRAM table; a sentinel row (al_src = -1e30) makes padded edge
  slots contribute exactly zero to the segment softmax.
- Per-edge work is batched over groups of blocks: one multi-row indirect DMA
  gathers thousands of table rows per instruction, and attention
  (leaky-relu, exp, weighting, segment sums) runs as a handful of 4D-AP DVE
  ops per group instead of per-128-edge-tile instructions.
"""
import sys
import types
import numpy as np
import concourse.bass as bass
import concourse.mybir as mybir
import concourse.tile as tile
from concourse import bacc
from concourse.bass_utils import run_bass_kernel_spmd


def _install_axon_ntff_shim():
    """Allow trace=True under axon when antenv.axon_hooks is absent."""
    if "antenv.axon_hooks" in sys.modules:
        return True
    try:
        from trn_agent_boot.trn_boot import _ntff_profile_via_ctypes
        hook = _ntff_profile_via_ctypes("/opt/axon/libaxon_pjrt.so")
        mod = types.ModuleType("antenv.axon_hooks")
        mod._hook = hook
        mod.set_axon_ntff_profile_hook = lambda h: setattr(mod, "_hook", h)
        mod.get_axon_ntff_profile_hook = lambda: mod._hook
        sys.modules["antenv.axon_hooks"] = mod
        import antenv
        antenv.axon_hooks = mod
        return True
    except Exception:
        return False


F32 = mybir.dt.float32
I32 = mybir.dt.int32
AX = mybir.AxisListType.X
NEG_SLOPE = 0.2
EPS = 1e-16

N_NODES = 100000
F_IN = 512
SHARD = 12544          # 98 blocks of 128 per core; 8*12544 = 100352
NB = 98
NPAD = 8 * SHARD
SLOTCAP = 448          # max edge-slot columns per group
GCHUNK = 64            # offset columns per indirect DMA (8192 descriptors)


class Cfg:
    def __init__(self, groups):
        # groups: list of (lb0, G, TG); consecutive blocks, same for all cores
        self.groups = groups
        self.NT = sum(G * TG for (_, G, TG) in groups)
        self.SLOTMAX = max(G * TG for (_, G, TG) in groups)
        self.GMAX = max(G for (_, G, TG) in groups)
        assert self.SLOTMAX <= 640, self.SLOTMAX
        assert self.NT <= 4608, self.NT


def build(cfg: Cfg):
    nc = bacc.Bacc("TRN2", target_bir_lowering=False, debug=False, num_devices=8)

    xT = nc.dram_tensor("xT", [F_IN, SHARD], F32, kind="ExternalInput")
    pack1 = nc.dram_tensor("pack1", [F_IN, 24], F32, kind="ExternalInput")
    pk2repD = nc.dram_tensor("pk2rep", [128, 192], F32, kind="ExternalInput")
    b1rep = nc.dram_tensor("b1rep", [128, 16], F32, kind="ExternalInput")
    b2rep = nc.dram_tensor("b2rep", [128, 4], F32, kind="ExternalInput")
    wfcrep = nc.dram_tensor("wfcrep", [128, 4], F32, kind="ExternalInput")
    offsD = nc.dram_tensor("offs", [128, cfg.NT], I32, kind="ExternalInput")
    smulD = nc.dram_tensor("smul", [128, NB], F32, kind="ExternalInput")
    yout = nc.dram_tensor("yout", [128, NB], F32, kind="ExternalOutput")

    bfc_const = None  # bfc folded on host into b2? no - applied as immediate

    with tile.TileContext(nc) as tc:
        with (
            tc.tile_pool(name="const", bufs=1) as cp,
            tc.tile_pool(name="ps2", bufs=2, space="PSUM") as ps2,
            tc.tile_pool(name="sb", bufs=1) as sb,
            tc.tile_pool(name="db", bufs=2) as db,
            tc.tile_pool(name="dram", bufs=1, space="DRAM") as dp,
        ):
            SLOTMAX, GMAX, NT = cfg.SLOTMAX, cfg.GMAX, cfg.NT

            # ---- persistent SBUF tiles ----
            offs = cp.tile([128, NT], I32)
            nc.sync.dma_start(out=offs[:], in_=offsD[:, :])
            pk1 = cp.tile([128, 96], F32)
            nc.sync.dma_start(
                out=pk1[:].rearrange("p (k j) -> p k j", j=24),
                in_=pack1[:, :].rearrange("(k p) j -> p k j", p=128),
            )
            pk2r = cp.tile([128, 192], F32)
            nc.sync.dma_start(out=pk2r[:], in_=pk2repD[:, :])
            b1c = cp.tile([128, 16], F32)
            nc.sync.dma_start(out=b1c[:], in_=b1rep[:, :])
            b2c = cp.tile([128, 4], F32)
            nc.sync.dma_start(out=b2c[:], in_=b2rep[:, :])
            wfcc = cp.tile([128, 4], F32)
            nc.sync.dma_start(out=wfcc[:], in_=wfcrep[:, :])
            smul = cp.tile([128, NB], F32)
            nc.sync.dma_start(out=smul[:], in_=smulD[:, :])
            adT1 = cp.tile([128, NB * 4], F32)
            adT2 = cp.tile([128, NB * 4], F32)
            h1sb = cp.tile([128, NB * 16], F32)
            ysb = cp.tile([128, NB], F32)

            # DRAM intermediates
            g1loc = dp.tile([SHARD, 20], F32)
            tbl1 = dp.tile([NPAD + 1, 20], F32)
            g2loc = dp.tile([SHARD, 8], F32)
            tbl2 = dp.tile([NPAD + 1, 8], F32)

            # sentinel rows (padded slots gather these; al_src=-1e30 => exp=0)
            sent1 = cp.tile([1, 20], F32)
            nc.vector.memset(sent1[:], 0.0)
            nc.vector.memset(sent1[:, 16:20], -1e30)
            nc.sync.dma_start(out=tbl1[NPAD : NPAD + 1, :], in_=sent1[:])
            sent2 = cp.tile([1, 8], F32)
            nc.vector.memset(sent2[:], 0.0)
            nc.vector.memset(sent2[:, 4:8], -1e30)
            nc.sync.dma_start(out=tbl2[NPAD : NPAD + 1, :], in_=sent2[:])

            # ---- P1: layer-1 node tables  g1 = x @ [W1 | W1A1s | W1A1d] ----
            SGS = [(0, 21), (21, 21), (42, 21), (63, 21), (84, 14)]
            for (b0, nsg) in SGS:
                acc = ps2.tile([128, 21 * 24], F32, tag="p1acc")
                xall = sb.tile([128, 4 * 21 * 128], F32, tag="xall")
                for k in range(4):
                    nc.sync.dma_start(
                        out=xall[:, k * 21 * 128 : k * 21 * 128 + nsg * 128],
                        in_=xT[k * 128 : (k + 1) * 128, b0 * 128 : (b0 + nsg) * 128],
                    )
                for i in range(nsg):
                    for k in range(4):
                        nc.tensor.matmul(
                            acc[:, i * 24 : (i + 1) * 24],
                            lhsT=xall[:, (k * 21 + i) * 128 : (k * 21 + i + 1) * 128],
                            rhs=pk1[:, k * 24 : (k + 1) * 24],
                            start=(k == 0),
                            stop=(k == 3),
                        )
                g1r = sb.tile([128, 21 * 24], F32, tag="g1r")
                nc.vector.tensor_copy(out=g1r[:, : nsg * 24], in_=acc[:, : nsg * 24])
                nc.sync.dma_start(
                    out=g1loc[b0 * 128 : (b0 + nsg) * 128, :].rearrange(
                        "(i p) j -> p i j", p=128
                    ),
                    in_=g1r[:, : nsg * 24].rearrange("p (i j) -> p i j", j=24)[
                        :, :, 0:20
                    ],
                )
                nc.vector.tensor_copy(
                    out=adT1[:, b0 * 4 : (b0 + nsg) * 4].rearrange(
                        "p (i j) -> p i j", j=4
                    ),
                    in_=g1r[:, : nsg * 24].rearrange("p (i j) -> p i j", j=24)[
                        :, :, 20:24
                    ],
                )

            # ---- P2: all-gather layer-1 table ----
            nc.gpsimd.collective_compute(
                "AllGather", mybir.AluOpType.bypass,
                replica_groups=[list(range(8))],
                ins=[g1loc.opt()], outs=[tbl1[0:NPAD, :].opt()],
            )

            # ---- P3: layer-1 edge phase, one pass per group ----
            def edge_phase(tbl, loc, nrow, adT, hcols, out_cb):
                """nrow: table row width; hcols: payload cols (h), then
                al_src at [hcols:hcols+4]. loc: core-local table (affine
                source for self-loop rows). out_cb(lb0, G, num, rec)
                consumes segment results: num [128,G*hcols], rec [128,G*4]."""
                cbase = 0
                for (lb0, G, TG) in cfg.groups:
                    cols = G * TG
                    gg = db.tile([128, SLOTMAX * 20], F32, tag="gg")
                    # one indirect DMA per 128-edge column: HW consumes one
                    # offset per partition-row and fetches a contiguous run
                    # of dest-size elements, so batching offset columns into
                    # a single DMA is not possible on this image.
                    for c in range(cols):
                        nc.gpsimd.indirect_dma_start(
                            out=gg[:, c * nrow : (c + 1) * nrow],
                            out_offset=None,
                            in_=tbl[:, :],
                            in_offset=bass.IndirectOffsetOnAxis(
                                ap=offs[:, cbase + c : cbase + c + 1], axis=0
                            ),
                        )
                    # views: (g, j, t) with j = col-in-row
                    gv = gg[:, : cols * nrow].rearrange(
                        "p (g t j) -> p g j t", t=TG, j=nrow
                    )
                    # t4 = al_src[src] + al_dst[dst]; layout (g, h, t)
                    adg = (
                        adT[:, lb0 * 4 : (lb0 + G) * 4]
                        .rearrange("p (g j) -> p g j", j=4)
                        .unsqueeze(3)
                        .broadcast_to([128, G, 4, TG])
                    )
                    t4 = sb.tile([128, SLOTMAX * 4], F32, tag="t4")
                    nc.vector.tensor_add(
                        out=t4[:, : cols * 4].rearrange(
                            "p (g j t) -> p g j t", j=4, t=TG
                        ),
                        in0=gv[:, :, hcols : hcols + 4, :],
                        in1=adg,
                    )
                    lr = sb.tile([128, SLOTMAX * 4], F32, tag="lr")
                    nc.vector.scalar_tensor_tensor(
                        out=lr[:, : cols * 4], in0=t4[:, : cols * 4],
                        scalar=NEG_SLOPE, in1=t4[:, : cols * 4],
                        op0=mybir.AluOpType.mult, op1=mybir.AluOpType.max,
                    )
                    ex = sb.tile([128, SLOTMAX * 4], F32, tag="ex")
                    nc.scalar.activation(
                        ex[:, : cols * 4], lr[:, : cols * 4],
                        mybir.ActivationFunctionType.Exp,
                    )
                    exv = ex[:, : cols * 4].rearrange(
                        "p (g h t) -> p g h t", h=4, t=TG
                    )
                    # denominator
                    den = sb.tile([128, GMAX * 4], F32, tag="den")
                    nc.vector.reduce_sum(out=den[:, : G * 4], in_=exv, axis=AX)
                    # weighted payload + numerator; CH = channels per head
                    CH = hcols // 4
                    payw = sb.tile([128, SLOTMAX * 16], F32, tag="payw")
                    if CH > 1:
                        pwv = payw[:, : cols * hcols].rearrange(
                            "p (g h c t) -> p g h c t", h=4, c=CH, t=TG
                        )
                        for h in range(4):
                            nc.vector.tensor_mul(
                                out=pwv[:, :, h : h + 1, :, :].squeeze(2),
                                in0=gv[:, :, h * CH : (h + 1) * CH, :],
                                in1=exv[:, :, h : h + 1, :].broadcast_to(
                                    [128, G, CH, TG]
                                ),
                            )
                    else:
                        nc.vector.tensor_mul(
                            out=payw[:, : cols * 4].rearrange(
                                "p (g h t) -> p g h t", h=4, t=TG
                            ),
                            in0=gv[:, :, 0:4, :],
                            in1=exv,
                        )
                    nums = sb.tile([128, GMAX * 16], F32, tag="nums")
                    nc.vector.reduce_sum(
                        out=nums[:, : G * hcols],
                        in_=payw[:, : cols * hcols].rearrange(
                            "p (q t) -> p q t", t=TG
                        ),
                        axis=AX,
                    )
                    # self-loop contribution: dst's own row, affine load,
                    # scaled by the self-edge multiplicity stream
                    selfb = sb.tile([128, GMAX * 20], F32, tag="selfb")
                    nc.sync.dma_start(
                        out=selfb[:, : G * nrow].rearrange(
                            "p (g j) -> p g j", j=nrow
                        ),
                        in_=loc[lb0 * 128 : (lb0 + G) * 128, :].rearrange(
                            "(g p) j -> p g j", p=128
                        ),
                    )
                    sv = selfb[:, : G * nrow].rearrange("p (g j) -> p g j", j=nrow)
                    t4s = sb.tile([128, GMAX * 4], F32, tag="t4s")
                    nc.vector.tensor_add(
                        out=t4s[:, : G * 4].rearrange("p (g j) -> p g j", j=4),
                        in0=sv[:, :, hcols : hcols + 4],
                        in1=adT[:, lb0 * 4 : (lb0 + G) * 4].rearrange(
                            "p (g j) -> p g j", j=4
                        ),
                    )
                    lrs = sb.tile([128, GMAX * 4], F32, tag="lrs")
                    nc.vector.scalar_tensor_tensor(
                        out=lrs[:, : G * 4], in0=t4s[:, : G * 4],
                        scalar=NEG_SLOPE, in1=t4s[:, : G * 4],
                        op0=mybir.AluOpType.mult, op1=mybir.AluOpType.max,
                    )
                    exs = sb.tile([128, GMAX * 4], F32, tag="exs")
                    nc.scalar.activation(
                        exs[:, : G * 4], lrs[:, : G * 4],
                        mybir.ActivationFunctionType.Exp,
                    )
                    nc.vector.tensor_mul(
                        out=exs[:, : G * 4].rearrange("p (g j) -> p g j", j=4),
                        in0=exs[:, : G * 4].rearrange("p (g j) -> p g j", j=4),
                        in1=smul[:, lb0 : lb0 + G]
                        .unsqueeze(2)
                        .broadcast_to([128, G, 4]),
                    )
                    nc.vector.tensor_add(
                        out=den[:, : G * 4], in0=den[:, : G * 4],
                        in1=exs[:, : G * 4],
                    )
                    pws = sb.tile([128, GMAX * 16], F32, tag="pws")
                    if hcols == 16:
                        nc.vector.tensor_mul(
                            out=pws[:, : G * 16].rearrange(
                                "p (g h c) -> p g h c", h=4, c=4
                            ),
                            in0=sv[:, :, 0:16].rearrange(
                                "p g (h c) -> p g h c", c=4
                            ),
                            in1=exs[:, : G * 4]
                            .rearrange("p (g h) -> p g h", h=4)
                            .unsqueeze(3)
                            .broadcast_to([128, G, 4, 4]),
                        )
                    else:
                        nc.vector.tensor_mul(
                            out=pws[:, : G * 4].rearrange("p (g j) -> p g j", j=4),
                            in0=sv[:, :, 0:4],
                            in1=exs[:, : G * 4].rearrange("p (g j) -> p g j", j=4),
                        )
                    nc.vector.tensor_add(
                        out=nums[:, : G * hcols], in0=nums[:, : G * hcols],
                        in1=pws[:, : G * hcols],
                    )
                    # reciprocal of denominator (+ one NR step)
                    sp = sb.tile([128, GMAX * 4], F32, tag="sp")
                    nc.vector.tensor_scalar_add(sp[:, : G * 4], den[:, : G * 4], EPS)
                    rec = sb.tile([128, GMAX * 4], F32, tag="rec")
                    nc.vector.reciprocal(rec[:, : G * 4], sp[:, : G * 4])
                    nr = sb.tile([128, GMAX * 4], F32, tag="nr")
                    nc.vector.tensor_mul(
                        out=nr[:, : G * 4], in0=sp[:, : G * 4], in1=rec[:, : G * 4]
                    )
                    nc.vector.tensor_scalar_mul(nr[:, : G * 4], nr[:, : G * 4], -1.0)
                    nc.vector.tensor_scalar_add(nr[:, : G * 4], nr[:, : G * 4], 2.0)
                    nc.vector.tensor_mul(
                        out=rec[:, : G * 4], in0=rec[:, : G * 4], in1=nr[:, : G * 4]
                    )
                    out_cb(lb0, G, nums, rec)
                    cbase += cols

            g2sb = cp.tile([128, NB * 12], F32)

            def l1_finalize(lb0, G, nums, rec):
                # h1 = elu(num*rec + b1)
                o16 = sb.tile([128, GMAX * 16], F32, tag="o16")
                nc.vector.tensor_mul(
                    out=o16[:, : G * 16].rearrange("p (g h c) -> p g h c", h=4, c=4),
                    in0=nums[:, : G * 16].rearrange("p (g h c) -> p g h c", h=4, c=4),
                    in1=rec[:, : G * 4]
                    .rearrange("p (g h) -> p g h", h=4)
                    .unsqueeze(3)
                    .broadcast_to([128, G, 4, 4]),
                )
                nc.vector.tensor_add(
                    out=o16[:, : G * 16].rearrange("p (g j) -> p g j", j=16),
                    in0=o16[:, : G * 16].rearrange("p (g j) -> p g j", j=16),
                    in1=b1c[:, :].unsqueeze(1).broadcast_to([128, G, 16]),
                )
                m0 = sb.tile([128, GMAX * 16], F32, tag="m0")
                nc.vector.tensor_scalar_min(m0[:, : G * 16], o16[:, : G * 16], 0.0)
                em = sb.tile([128, GMAX * 16], F32, tag="em")
                nc.scalar.activation(
                    em[:, : G * 16], m0[:, : G * 16],
                    mybir.ActivationFunctionType.Exp,
                )
                nc.vector.tensor_scalar_add(em[:, : G * 16], em[:, : G * 16], -1.0)
                nc.vector.tensor_tensor(
                    out=h1sb[:, lb0 * 16 : (lb0 + G) * 16],
                    in0=o16[:, : G * 16],
                    in1=em[:, : G * 16],
                    op=mybir.AluOpType.max,
                )
                # fused P4: layer-2 table rows for this group (overlaps the
                # remaining L1 gather stream instead of running serially)
                for j in range(12):
                    tmp = sb.tile([128, GMAX * 16], F32, tag="p4tmp")
                    nc.vector.tensor_mul(
                        out=tmp[:, : G * 16].rearrange("p (b k) -> p b k", k=16),
                        in0=h1sb[:, lb0 * 16 : (lb0 + G) * 16].rearrange(
                            "p (b k) -> p b k", k=16
                        ),
                        in1=pk2r[:, j * 16 : (j + 1) * 16]
                        .unsqueeze(1)
                        .broadcast_to([128, G, 16]),
                    )
                    nc.vector.reduce_sum(
                        out=g2sb[:, lb0 * 12 : (lb0 + G) * 12].rearrange(
                            "p (b j) -> p b j", j=12
                        )[:, :, j : j + 1],
                        in_=tmp[:, : G * 16].rearrange("p (b k) -> p b k", k=16),
                        axis=AX,
                    )
                nc.vector.tensor_copy(
                    out=adT2[:, lb0 * 4 : (lb0 + G) * 4].rearrange(
                        "p (b j) -> p b j", j=4
                    ),
                    in_=g2sb[:, lb0 * 12 : (lb0 + G) * 12].rearrange(
                        "p (b j) -> p b j", j=12
                    )[:, :, 8:12],
                )
                nc.sync.dma_start(
                    out=g2loc[lb0 * 128 : (lb0 + G) * 128, :].rearrange(
                        "(b p) j -> p b j", p=128
                    ),
                    in_=g2sb[:, lb0 * 12 : (lb0 + G) * 12].rearrange(
                        "p (b j) -> p b j", j=12
                    )[:, :, 0:8],
                )

            edge_phase(tbl1, g1loc, 20, adT1, 16, l1_finalize)

            # ---- P5: all-gather layer-2 table ----
            nc.gpsimd.collective_compute(
                "AllGather", mybir.AluOpType.bypass,
                replica_groups=[list(range(8))],
                ins=[g2loc.opt()], outs=[tbl2[0:NPAD, :].opt()],
            )

            # ---- P6: layer-2 edge phase + fc head ----
            def l2_finalize(lb0, G, nums, rec):
                o4 = sb.tile([128, GMAX * 4], F32, tag="o4")
                nc.vector.tensor_mul(
                    out=o4[:, : G * 4], in0=nums[:, : G * 4], in1=rec[:, : G * 4]
                )
                nc.vector.tensor_add(
                    out=o4[:, : G * 4].rearrange("p (g j) -> p g j", j=4),
                    in0=o4[:, : G * 4].rearrange("p (g j) -> p g j", j=4),
                    in1=b2c[:, :].unsqueeze(1).broadcast_to([128, G, 4]),
                )
                nc.vector.tensor_mul(
                    out=o4[:, : G * 4].rearrange("p (g j) -> p g j", j=4),
                    in0=o4[:, : G * 4].rearrange("p (g j) -> p g j", j=4),
                    in1=wfcc[:, :].unsqueeze(1).broadcast_to([128, G, 4]),
                )
                nc.vector.reduce_sum(
                    out=ysb[:, lb0 : lb0 + G],
                    in_=o4[:, : G * 4].rearrange("p (g j) -> p g j", j=4),
                    axis=AX,
                )

            edge_phase(tbl2, g2loc, 8, adT2, 4, l2_finalize)

            nc.sync.dma_start(out=yout[:, :], in_=ysb[:])
    nc.compile()
    return nc


def compute_groups(T_lb):
    """T_lb: [NB] per-octet tile heights (non-increasing). Returns group list."""
    groups = []
    lb = 0
    while lb < NB:
        TG = max(int(T_lb[lb]), 1)
        G = min(NB - lb, max(1, SLOTCAP // TG))
        groups.append((lb, G, TG))
        lb += G
    return groups


def host_prep(inputs: dict):
    x = np.asarray(inputs["x"], np.float32)
    ei = np.asarray(inputs["edge_index"])
    src = np.concatenate([ei[0], np.arange(N_NODES, dtype=np.int64)]).astype(np.int64)
    dst = np.concatenate([ei[1], np.arange(N_NODES, dtype=np.int64)]).astype(np.int64)

    # self edges (added loops + natural (i,i)) go through the affine path
    selfmask = src == dst
    m = np.bincount(dst[selfmask], minlength=NPAD).astype(np.float32)
    src = src[~selfmask]
    dst = dst[~selfmask]

    deg = np.bincount(dst, minlength=NPAD)
    order = np.argsort(-deg, kind="stable")           # new_id -> old_id
    inv = np.empty(NPAD, np.int64)
    inv[order] = np.arange(NPAD)
    degs = deg[order]
    morder = m[order]                                 # self multiplicity

    T_lb = degs[np.arange(NB) * 1024]                 # octet max degrees
    groups = compute_groups(T_lb)
    cfg = Cfg(groups)

    colarr = np.zeros(NB, np.int64)                   # per-block column base
    base = 0
    for (lb0, G, TG) in groups:
        for i in range(G):
            colarr[lb0 + i] = base + i * TG
        base += G * TG
    assert base == cfg.NT

    nd = inv[dst]
    ns = inv[src]
    eorder = np.argsort(nd, kind="stable")
    nd_s = nd[eorder]
    ns_s = ns[eorder]
    E = len(nd_s)
    starts = np.zeros(NPAD + 1, np.int64)
    np.cumsum(np.bincount(nd_s, minlength=NPAD), out=starts[1:])
    rank = np.arange(E, dtype=np.int64) - starts[nd_s]

    g_d = nd_s // 128
    core_s = g_d % 8
    lb_s = g_d // 8
    p_s = nd_s % 128
    gsrc = ns_s // 128
    trow = ((gsrc % 8) * SHARD + (gsrc // 8) * 128 + (ns_s % 128)).astype(np.int64)
    col = colarr[lb_s] + rank

    offs_all = np.full((8, 128, cfg.NT), NPAD, np.int32)
    offs_all[core_s, p_s, col] = trow.astype(np.int32)

    # weight packs (host precompute)
    W1 = np.asarray(inputs["W1"], np.float32)
    a_src1 = np.asarray(inputs["a_src1"], np.float32)
    a_dst1 = np.asarray(inputs["a_dst1"], np.float32)
    A1s = np.zeros((16, 4), np.float32)
    A1d = np.zeros((16, 4), np.float32)
    for h in range(4):
        A1s[h * 4 : h * 4 + 4, h] = a_src1[h]
        A1d[h * 4 : h * 4 + 4, h] = a_dst1[h]
    pack1 = np.concatenate([W1, W1 @ A1s, W1 @ A1d], axis=1)  # [512, 24]

    W2 = np.asarray(inputs["W2"], np.float32)
    a2s = np.asarray(inputs["a_src2"], np.float32)[:, 0]
    a2d = np.asarray(inputs["a_dst2"], np.float32)[:, 0]
    pack2 = np.concatenate([W2, W2 * a2s[None, :], W2 * a2d[None, :]], axis=1)  # [16,12]
    pk2rep = np.tile(pack2.T.reshape(1, 192), (128, 1)).astype(np.float32)

    b1rep = np.tile(np.asarray(inputs["b1"], np.float32)[None, :], (128, 1))
    b2rep = np.tile(np.asarray(inputs["b2"], np.float32)[None, :], (128, 1))
    wfcrep = np.tile(np.asarray(inputs["Wfc"], np.float32)[:, 0][None, :], (128, 1))
    bfc = float(np.asarray(inputs["bfc"])[0])

    xp = np.zeros((NPAD, F_IN), np.float32)
    vm = order < N_NODES
    xp[vm] = x[order[vm]]
    xpb = xp.reshape(784, 128, F_IN)

    mb = morder.reshape(784, 128)
    in_maps = []
    for c in range(8):
        xT_c = np.ascontiguousarray(
            xpb[c::8].reshape(SHARD, F_IN).T
        )
        smul_c = np.ascontiguousarray(mb[c::8].T)     # [128, NB]
        in_maps.append({
            "xT": xT_c, "pack1": pack1, "pk2rep": pk2rep,
            "b1rep": b1rep, "b2rep": b2rep, "wfcrep": wfcrep,
            "offs": np.ascontiguousarray(offs_all[c]),
            "smul": smul_c,
        })
    return cfg, in_maps, order, vm, bfc


def assemble_output(results, order, vm, bfc):
    ynew = np.zeros(NPAD, np.float32)
    yb = ynew.reshape(784, 128)
    for c in range(8):
        yb[c::8] = np.asarray(results[c]["yout"]).T  # [NB,128]
    y = np.empty(N_NODES, np.float32)
    y[order[vm]] = ynew[vm] + bfc
    return y[:, None]


LAST_EXEC_NS = None


def run(inputs: dict, trace: bool = False):
    cfg, in_maps, order, vm, bfc = host_prep(inputs)
    nc = build(cfg)
    res = run_bass_kernel_spmd(nc, in_maps, core_ids=list(range(8)), trace=trace)
    y = assemble_output(res.results, order, vm, bfc)
    return y, res


def kernel(**inputs) -> np.ndarray:
    global LAST_EXEC_NS
    trace = _install_axon_ntff_shim()
    try:
        y, res = run(inputs, trace=trace)
    except Exception:
        if not trace:
            raise
        y, res = run(inputs, trace=False)
    LAST_EXEC_NS = res.exec_time_ns
    return np.ascontiguousarray(y.astype(np.float32))


# revision 22
# speedup vs baseline: 1.0785x; 1.0748x over previous
"""Two-layer GAT (PyG GATConv x2 + linear head) on 8 TRN2 NeuronCores via Bass/Tile.

Self-contained kernel: kernel(**inputs) takes the FULL inputs of
reference.setup_inputs() and returns the FULL [100000, 1] float32 output.

Strategy (degree-sorted dst sharding, fully batched):
- Host sorts nodes by in-degree and deals 128-node blocks round-robin to the
  8 cores; every block's edge slots form a dense [128, Tb] grid (Tb = max
  degree in the block's octet) so there is NO overflow path at all.
- Node tables [h | al_src] are built per-core with matmuls and AllGathered to
  a replicated DRAM table; a sentinel row (al_src = -1e30) makes padded edge
  slots contribute exactly zero to the segment softmax.
- Per-edge work is batched over groups of blocks: one multi-row indirect DMA
  gathers thousands of table rows per instruction, and attention
  (leaky-relu, exp, weighting, segment sums) runs as a handful of 4D-AP DVE
  ops per group instead of per-128-edge-tile instructions.
"""
import sys
import types
import numpy as np
import concourse.bass as bass
import concourse.mybir as mybir
import concourse.tile as tile
from concourse import bacc
from concourse.bass_utils import run_bass_kernel_spmd


def _install_axon_ntff_shim():
    """Allow trace=True under axon when antenv.axon_hooks is absent."""
    if "antenv.axon_hooks" in sys.modules:
        return True
    try:
        from trn_agent_boot.trn_boot import _ntff_profile_via_ctypes
        hook = _ntff_profile_via_ctypes("/opt/axon/libaxon_pjrt.so")
        mod = types.ModuleType("antenv.axon_hooks")
        mod._hook = hook
        mod.set_axon_ntff_profile_hook = lambda h: setattr(mod, "_hook", h)
        mod.get_axon_ntff_profile_hook = lambda: mod._hook
        sys.modules["antenv.axon_hooks"] = mod
        import antenv
        antenv.axon_hooks = mod
        return True
    except Exception:
        return False


F32 = mybir.dt.float32
I32 = mybir.dt.int32
AX = mybir.AxisListType.X
NEG_SLOPE = 0.2
EPS = 1e-16

N_NODES = 100000
F_IN = 512
SHARD = 12544          # 98 blocks of 128 per core; 8*12544 = 100352
NB = 98
NPAD = 8 * SHARD
SLOTCAP = 128          # small groups track the degree curve tightly (fewer gather cols)
GCHUNK = 64            # offset columns per indirect DMA (8192 descriptors)


class Cfg:
    def __init__(self, groups):
        # groups: list of (lb0, G, TG); consecutive blocks, same for all cores
        self.groups = groups
        self.NT = sum(G * TG for (_, G, TG) in groups)
        self.SLOTMAX = max(G * TG for (_, G, TG) in groups)
        self.GMAX = max(G for (_, G, TG) in groups)
        assert self.SLOTMAX <= 640, self.SLOTMAX
        assert self.NT <= 4608, self.NT


def build(cfg: Cfg):
    nc = bacc.Bacc("TRN2", target_bir_lowering=False, debug=False, num_devices=8)

    xT = nc.dram_tensor("xT", [F_IN, SHARD], F32, kind="ExternalInput")
    pack1 = nc.dram_tensor("pack1", [F_IN, 24], F32, kind="ExternalInput")
    pk2repD = nc.dram_tensor("pk2rep", [128, 192], F32, kind="ExternalInput")
    b1rep = nc.dram_tensor("b1rep", [128, 16], F32, kind="ExternalInput")
    b2rep = nc.dram_tensor("b2rep", [128, 4], F32, kind="ExternalInput")
    wfcrep = nc.dram_tensor("wfcrep", [128, 4], F32, kind="ExternalInput")
    offsD = nc.dram_tensor("offs", [128, cfg.NT], I32, kind="ExternalInput")
    smulD = nc.dram_tensor("smul", [128, NB], F32, kind="ExternalInput")
    yout = nc.dram_tensor("yout", [128, NB], F32, kind="ExternalOutput")

    bfc_const = None  # bfc folded on host into b2? no - applied as immediate

    with tile.TileContext(nc) as tc:
        with (
            tc.tile_pool(name="const", bufs=1) as cp,
            tc.tile_pool(name="ps2", bufs=2, space="PSUM") as ps2,
            tc.tile_pool(name="sb", bufs=1) as sb,
            tc.tile_pool(name="db", bufs=2) as db,
            tc.tile_pool(name="dram", bufs=1, space="DRAM") as dp,
        ):
            SLOTMAX, GMAX, NT = cfg.SLOTMAX, cfg.GMAX, cfg.NT

            # ---- persistent SBUF tiles ----
            offs = cp.tile([128, NT], I32)
            nc.sync.dma_start(out=offs[:], in_=offsD[:, :])
            pk1 = cp.tile([128, 96], F32)
            nc.sync.dma_start(
                out=pk1[:].rearrange("p (k j) -> p k j", j=24),
                in_=pack1[:, :].rearrange("(k p) j -> p k j", p=128),
            )
            pk2r = cp.tile([128, 192], F32)
            nc.sync.dma_start(out=pk2r[:], in_=pk2repD[:, :])
            b1c = cp.tile([128, 16], F32)
            nc.sync.dma_start(out=b1c[:], in_=b1rep[:, :])
            b2c = cp.tile([128, 4], F32)
            nc.sync.dma_start(out=b2c[:], in_=b2rep[:, :])
            wfcc = cp.tile([128, 4], F32)
            nc.sync.dma_start(out=wfcc[:], in_=wfcrep[:, :])
            smul = cp.tile([128, NB], F32)
            nc.sync.dma_start(out=smul[:], in_=smulD[:, :])
            adT1 = cp.tile([128, NB * 4], F32)
            adT2 = cp.tile([128, NB * 4], F32)
            h1sb = cp.tile([128, NB * 16], F32)
            ysb = cp.tile([128, NB], F32)

            # DRAM intermediates
            g1loc = dp.tile([SHARD, 20], F32)
            tbl1 = dp.tile([NPAD + 1, 20], F32)
            g2loc = dp.tile([SHARD, 8], F32)
            tbl2 = dp.tile([NPAD + 1, 8], F32)

            # sentinel rows (padded slots gather these; al_src=-1e30 => exp=0)
            sent1 = cp.tile([1, 20], F32)
            nc.vector.memset(sent1[:], 0.0)
            nc.vector.memset(sent1[:, 16:20], -1e30)
            nc.sync.dma_start(out=tbl1[NPAD : NPAD + 1, :], in_=sent1[:])
            sent2 = cp.tile([1, 8], F32)
            nc.vector.memset(sent2[:], 0.0)
            nc.vector.memset(sent2[:, 4:8], -1e30)
            nc.sync.dma_start(out=tbl2[NPAD : NPAD + 1, :], in_=sent2[:])

            # ---- P1: layer-1 node tables  g1 = x @ [W1 | W1A1s | W1A1d] ----
            SGS = [(0, 21), (21, 21), (42, 21), (63, 21), (84, 14)]
            for (b0, nsg) in SGS:
                acc = ps2.tile([128, 21 * 24], F32, tag="p1acc")
                xall = sb.tile([128, 4 * 21 * 128], F32, tag="xall")
                for k in range(4):
                    nc.sync.dma_start(
                        out=xall[:, k * 21 * 128 : k * 21 * 128 + nsg * 128],
                        in_=xT[k * 128 : (k + 1) * 128, b0 * 128 : (b0 + nsg) * 128],
                    )
                for i in range(nsg):
                    for k in range(4):
                        nc.tensor.matmul(
                            acc[:, i * 24 : (i + 1) * 24],
                            lhsT=xall[:, (k * 21 + i) * 128 : (k * 21 + i + 1) * 128],
                            rhs=pk1[:, k * 24 : (k + 1) * 24],
                            start=(k == 0),
                            stop=(k == 3),
                        )
                g1r = sb.tile([128, 21 * 24], F32, tag="g1r")
                nc.vector.tensor_copy(out=g1r[:, : nsg * 24], in_=acc[:, : nsg * 24])
                nc.sync.dma_start(
                    out=g1loc[b0 * 128 : (b0 + nsg) * 128, :].rearrange(
                        "(i p) j -> p i j", p=128
                    ),
                    in_=g1r[:, : nsg * 24].rearrange("p (i j) -> p i j", j=24)[
                        :, :, 0:20
                    ],
                )
                nc.vector.tensor_copy(
                    out=adT1[:, b0 * 4 : (b0 + nsg) * 4].rearrange(
                        "p (i j) -> p i j", j=4
                    ),
                    in_=g1r[:, : nsg * 24].rearrange("p (i j) -> p i j", j=24)[
                        :, :, 20:24
                    ],
                )

            # ---- P2: all-gather layer-1 table ----
            nc.gpsimd.collective_compute(
                "AllGather", mybir.AluOpType.bypass,
                replica_groups=[list(range(8))],
                ins=[g1loc.opt()], outs=[tbl1[0:NPAD, :].opt()],
            )

            # ---- P3: layer-1 edge phase, one pass per group ----
            def edge_phase(tbl, loc, nrow, adT, hcols, out_cb):
                """nrow: table row width; hcols: payload cols (h), then
                al_src at [hcols:hcols+4]. loc: core-local table (affine
                source for self-loop rows). out_cb(lb0, G, num, rec)
                consumes segment results: num [128,G*hcols], rec [128,G*4]."""
                cbase = 0
                for (lb0, G, TG) in cfg.groups:
                    cols = G * TG
                    gg = db.tile([128, SLOTMAX * 20], F32, tag="gg")
                    # one indirect DMA per 128-edge column: HW consumes one
                    # offset per partition-row and fetches a contiguous run
                    # of dest-size elements, so batching offset columns into
                    # a single DMA is not possible on this image.
                    for c in range(cols):
                        nc.gpsimd.indirect_dma_start(
                            out=gg[:, c * nrow : (c + 1) * nrow],
                            out_offset=None,
                            in_=tbl[:, :],
                            in_offset=bass.IndirectOffsetOnAxis(
                                ap=offs[:, cbase + c : cbase + c + 1], axis=0
                            ),
                        )
                    # views: (g, j, t) with j = col-in-row
                    gv = gg[:, : cols * nrow].rearrange(
                        "p (g t j) -> p g j t", t=TG, j=nrow
                    )
                    # t4 = al_src[src] + al_dst[dst]; layout (g, h, t)
                    adg = (
                        adT[:, lb0 * 4 : (lb0 + G) * 4]
                        .rearrange("p (g j) -> p g j", j=4)
                        .unsqueeze(3)
                        .broadcast_to([128, G, 4, TG])
                    )
                    t4 = sb.tile([128, SLOTMAX * 4], F32, tag="t4")
                    nc.vector.tensor_add(
                        out=t4[:, : cols * 4].rearrange(
                            "p (g j t) -> p g j t", j=4, t=TG
                        ),
                        in0=gv[:, :, hcols : hcols + 4, :],
                        in1=adg,
                    )
                    lr = sb.tile([128, SLOTMAX * 4], F32, tag="lr")
                    nc.vector.scalar_tensor_tensor(
                        out=lr[:, : cols * 4], in0=t4[:, : cols * 4],
                        scalar=NEG_SLOPE, in1=t4[:, : cols * 4],
                        op0=mybir.AluOpType.mult, op1=mybir.AluOpType.max,
                    )
                    ex = sb.tile([128, SLOTMAX * 4], F32, tag="ex")
                    nc.scalar.activation(
                        ex[:, : cols * 4], lr[:, : cols * 4],
                        mybir.ActivationFunctionType.Exp,
                    )
                    exv = ex[:, : cols * 4].rearrange(
                        "p (g h t) -> p g h t", h=4, t=TG
                    )
                    # denominator
                    den = sb.tile([128, GMAX * 4], F32, tag="den")
                    nc.vector.reduce_sum(out=den[:, : G * 4], in_=exv, axis=AX)
                    # weighted payload + numerator; CH = channels per head
                    CH = hcols // 4
                    payw = sb.tile([128, SLOTMAX * 16], F32, tag="payw")
                    if CH > 1:
                        pwv = payw[:, : cols * hcols].rearrange(
                            "p (g h c t) -> p g h c t", h=4, c=CH, t=TG
                        )
                        for h in range(4):
                            nc.vector.tensor_mul(
                                out=pwv[:, :, h : h + 1, :, :].squeeze(2),
                                in0=gv[:, :, h * CH : (h + 1) * CH, :],
                                in1=exv[:, :, h : h + 1, :].broadcast_to(
                                    [128, G, CH, TG]
                                ),
                            )
                    else:
                        nc.vector.tensor_mul(
                            out=payw[:, : cols * 4].rearrange(
                                "p (g h t) -> p g h t", h=4, t=TG
                            ),
                            in0=gv[:, :, 0:4, :],
                            in1=exv,
                        )
                    nums = sb.tile([128, GMAX * 16], F32, tag="nums")
                    nc.vector.reduce_sum(
                        out=nums[:, : G * hcols],
                        in_=payw[:, : cols * hcols].rearrange(
                            "p (q t) -> p q t", t=TG
                        ),
                        axis=AX,
                    )
                    # self-loop contribution: dst's own row, affine load,
                    # scaled by the self-edge multiplicity stream
                    selfb = sb.tile([128, GMAX * 20], F32, tag="selfb")
                    nc.sync.dma_start(
                        out=selfb[:, : G * nrow].rearrange(
                            "p (g j) -> p g j", j=nrow
                        ),
                        in_=loc[lb0 * 128 : (lb0 + G) * 128, :].rearrange(
                            "(g p) j -> p g j", p=128
                        ),
                    )
                    sv = selfb[:, : G * nrow].rearrange("p (g j) -> p g j", j=nrow)
                    t4s = sb.tile([128, GMAX * 4], F32, tag="t4s")
                    nc.vector.tensor_add(
                        out=t4s[:, : G * 4].rearrange("p (g j) -> p g j", j=4),
                        in0=sv[:, :, hcols : hcols + 4],
                        in1=adT[:, lb0 * 4 : (lb0 + G) * 4].rearrange(
                            "p (g j) -> p g j", j=4
                        ),
                    )
                    lrs = sb.tile([128, GMAX * 4], F32, tag="lrs")
                    nc.vector.scalar_tensor_tensor(
                        out=lrs[:, : G * 4], in0=t4s[:, : G * 4],
                        scalar=NEG_SLOPE, in1=t4s[:, : G * 4],
                        op0=mybir.AluOpType.mult, op1=mybir.AluOpType.max,
                    )
                    exs = sb.tile([128, GMAX * 4], F32, tag="exs")
                    nc.scalar.activation(
                        exs[:, : G * 4], lrs[:, : G * 4],
                        mybir.ActivationFunctionType.Exp,
                    )
                    nc.vector.tensor_mul(
                        out=exs[:, : G * 4].rearrange("p (g j) -> p g j", j=4),
                        in0=exs[:, : G * 4].rearrange("p (g j) -> p g j", j=4),
                        in1=smul[:, lb0 : lb0 + G]
                        .unsqueeze(2)
                        .broadcast_to([128, G, 4]),
                    )
                    nc.vector.tensor_add(
                        out=den[:, : G * 4], in0=den[:, : G * 4],
                        in1=exs[:, : G * 4],
                    )
                    pws = sb.tile([128, GMAX * 16], F32, tag="pws")
                    if hcols == 16:
                        nc.vector.tensor_mul(
                            out=pws[:, : G * 16].rearrange(
                                "p (g h c) -> p g h c", h=4, c=4
                            ),
                            in0=sv[:, :, 0:16].rearrange(
                                "p g (h c) -> p g h c", c=4
                            ),
                            in1=exs[:, : G * 4]
                            .rearrange("p (g h) -> p g h", h=4)
                            .unsqueeze(3)
                            .broadcast_to([128, G, 4, 4]),
                        )
                    else:
                        nc.vector.tensor_mul(
                            out=pws[:, : G * 4].rearrange("p (g j) -> p g j", j=4),
                            in0=sv[:, :, 0:4],
                            in1=exs[:, : G * 4].rearrange("p (g j) -> p g j", j=4),
                        )
                    nc.vector.tensor_add(
                        out=nums[:, : G * hcols], in0=nums[:, : G * hcols],
                        in1=pws[:, : G * hcols],
                    )
                    # reciprocal of denominator (+ one NR step)
                    sp = sb.tile([128, GMAX * 4], F32, tag="sp")
                    nc.vector.tensor_scalar_add(sp[:, : G * 4], den[:, : G * 4], EPS)
                    rec = sb.tile([128, GMAX * 4], F32, tag="rec")
                    nc.vector.reciprocal(rec[:, : G * 4], sp[:, : G * 4])
                    nr = sb.tile([128, GMAX * 4], F32, tag="nr")
                    nc.vector.tensor_mul(
                        out=nr[:, : G * 4], in0=sp[:, : G * 4], in1=rec[:, : G * 4]
                    )
                    nc.vector.tensor_scalar_mul(nr[:, : G * 4], nr[:, : G * 4], -1.0)
                    nc.vector.tensor_scalar_add(nr[:, : G * 4], nr[:, : G * 4], 2.0)
                    nc.vector.tensor_mul(
                        out=rec[:, : G * 4], in0=rec[:, : G * 4], in1=nr[:, : G * 4]
                    )
                    out_cb(lb0, G, nums, rec)
                    cbase += cols

            g2sb = cp.tile([128, NB * 12], F32)

            def l1_finalize(lb0, G, nums, rec):
                # h1 = elu(num*rec + b1)
                o16 = sb.tile([128, GMAX * 16], F32, tag="o16")
                nc.vector.tensor_mul(
                    out=o16[:, : G * 16].rearrange("p (g h c) -> p g h c", h=4, c=4),
                    in0=nums[:, : G * 16].rearrange("p (g h c) -> p g h c", h=4, c=4),
                    in1=rec[:, : G * 4]
                    .rearrange("p (g h) -> p g h", h=4)
                    .unsqueeze(3)
                    .broadcast_to([128, G, 4, 4]),
                )
                nc.vector.tensor_add(
                    out=o16[:, : G * 16].rearrange("p (g j) -> p g j", j=16),
                    in0=o16[:, : G * 16].rearrange("p (g j) -> p g j", j=16),
                    in1=b1c[:, :].unsqueeze(1).broadcast_to([128, G, 16]),
                )
                m0 = sb.tile([128, GMAX * 16], F32, tag="m0")
                nc.vector.tensor_scalar_min(m0[:, : G * 16], o16[:, : G * 16], 0.0)
                em = sb.tile([128, GMAX * 16], F32, tag="em")
                nc.scalar.activation(
                    em[:, : G * 16], m0[:, : G * 16],
                    mybir.ActivationFunctionType.Exp,
                )
                nc.vector.tensor_scalar_add(em[:, : G * 16], em[:, : G * 16], -1.0)
                nc.vector.tensor_tensor(
                    out=h1sb[:, lb0 * 16 : (lb0 + G) * 16],
                    in0=o16[:, : G * 16],
                    in1=em[:, : G * 16],
                    op=mybir.AluOpType.max,
                )
                # fused P4: layer-2 table rows for this group (overlaps the
                # remaining L1 gather stream instead of running serially)
                for j in range(12):
                    tmp = sb.tile([128, GMAX * 16], F32, tag="p4tmp")
                    nc.vector.tensor_mul(
                        out=tmp[:, : G * 16].rearrange("p (b k) -> p b k", k=16),
                        in0=h1sb[:, lb0 * 16 : (lb0 + G) * 16].rearrange(
                            "p (b k) -> p b k", k=16
                        ),
                        in1=pk2r[:, j * 16 : (j + 1) * 16]
                        .unsqueeze(1)
                        .broadcast_to([128, G, 16]),
                    )
                    nc.vector.reduce_sum(
                        out=g2sb[:, lb0 * 12 : (lb0 + G) * 12].rearrange(
                            "p (b j) -> p b j", j=12
                        )[:, :, j : j + 1],
                        in_=tmp[:, : G * 16].rearrange("p (b k) -> p b k", k=16),
                        axis=AX,
                    )
                nc.vector.tensor_copy(
                    out=adT2[:, lb0 * 4 : (lb0 + G) * 4].rearrange(
                        "p (b j) -> p b j", j=4
                    ),
                    in_=g2sb[:, lb0 * 12 : (lb0 + G) * 12].rearrange(
                        "p (b j) -> p b j", j=12
                    )[:, :, 8:12],
                )
                nc.sync.dma_start(
                    out=g2loc[lb0 * 128 : (lb0 + G) * 128, :].rearrange(
                        "(b p) j -> p b j", p=128
                    ),
                    in_=g2sb[:, lb0 * 12 : (lb0 + G) * 12].rearrange(
                        "p (b j) -> p b j", j=12
                    )[:, :, 0:8],
                )

            edge_phase(tbl1, g1loc, 20, adT1, 16, l1_finalize)

            # ---- P5: all-gather layer-2 table ----
            nc.gpsimd.collective_compute(
                "AllGather", mybir.AluOpType.bypass,
                replica_groups=[list(range(8))],
                ins=[g2loc.opt()], outs=[tbl2[0:NPAD, :].opt()],
            )

            # ---- P6: layer-2 edge phase + fc head ----
            def l2_finalize(lb0, G, nums, rec):
                o4 = sb.tile([128, GMAX * 4], F32, tag="o4")
                nc.vector.tensor_mul(
                    out=o4[:, : G * 4], in0=nums[:, : G * 4], in1=rec[:, : G * 4]
                )
                nc.vector.tensor_add(
                    out=o4[:, : G * 4].rearrange("p (g j) -> p g j", j=4),
                    in0=o4[:, : G * 4].rearrange("p (g j) -> p g j", j=4),
                    in1=b2c[:, :].unsqueeze(1).broadcast_to([128, G, 4]),
                )
                nc.vector.tensor_mul(
                    out=o4[:, : G * 4].rearrange("p (g j) -> p g j", j=4),
                    in0=o4[:, : G * 4].rearrange("p (g j) -> p g j", j=4),
                    in1=wfcc[:, :].unsqueeze(1).broadcast_to([128, G, 4]),
                )
                nc.vector.reduce_sum(
                    out=ysb[:, lb0 : lb0 + G],
                    in_=o4[:, : G * 4].rearrange("p (g j) -> p g j", j=4),
                    axis=AX,
                )

            edge_phase(tbl2, g2loc, 8, adT2, 4, l2_finalize)

            nc.sync.dma_start(out=yout[:, :], in_=ysb[:])
    nc.compile()
    return nc


def compute_groups(T_lb):
    """T_lb: [NB] per-octet tile heights (non-increasing). Returns group list."""
    groups = []
    lb = 0
    while lb < NB:
        TG = max(int(T_lb[lb]), 1)
        G = min(NB - lb, max(1, SLOTCAP // TG))
        groups.append((lb, G, TG))
        lb += G
    return groups


def host_prep(inputs: dict):
    x = np.asarray(inputs["x"], np.float32)
    ei = np.asarray(inputs["edge_index"])
    src = np.concatenate([ei[0], np.arange(N_NODES, dtype=np.int64)]).astype(np.int64)
    dst = np.concatenate([ei[1], np.arange(N_NODES, dtype=np.int64)]).astype(np.int64)

    # self edges (added loops + natural (i,i)) go through the affine path
    selfmask = src == dst
    m = np.bincount(dst[selfmask], minlength=NPAD).astype(np.float32)
    src = src[~selfmask]
    dst = dst[~selfmask]

    deg = np.bincount(dst, minlength=NPAD)
    order = np.argsort(-deg, kind="stable")           # new_id -> old_id
    inv = np.empty(NPAD, np.int64)
    inv[order] = np.arange(NPAD)
    degs = deg[order]
    morder = m[order]                                 # self multiplicity

    T_lb = degs[np.arange(NB) * 1024]                 # octet max degrees
    groups = compute_groups(T_lb)
    cfg = Cfg(groups)

    colarr = np.zeros(NB, np.int64)                   # per-block column base
    base = 0
    for (lb0, G, TG) in groups:
        for i in range(G):
            colarr[lb0 + i] = base + i * TG
        base += G * TG
    assert base == cfg.NT

    nd = inv[dst]
    ns = inv[src]
    eorder = np.argsort(nd, kind="stable")
    nd_s = nd[eorder]
    ns_s = ns[eorder]
    E = len(nd_s)
    starts = np.zeros(NPAD + 1, np.int64)
    np.cumsum(np.bincount(nd_s, minlength=NPAD), out=starts[1:])
    rank = np.arange(E, dtype=np.int64) - starts[nd_s]

    g_d = nd_s // 128
    core_s = g_d % 8
    lb_s = g_d // 8
    p_s = nd_s % 128
    gsrc = ns_s // 128
    trow = ((gsrc % 8) * SHARD + (gsrc // 8) * 128 + (ns_s % 128)).astype(np.int64)
    col = colarr[lb_s] + rank

    offs_all = np.full((8, 128, cfg.NT), NPAD, np.int32)
    offs_all[core_s, p_s, col] = trow.astype(np.int32)

    # weight packs (host precompute)
    W1 = np.asarray(inputs["W1"], np.float32)
    a_src1 = np.asarray(inputs["a_src1"], np.float32)
    a_dst1 = np.asarray(inputs["a_dst1"], np.float32)
    A1s = np.zeros((16, 4), np.float32)
    A1d = np.zeros((16, 4), np.float32)
    for h in range(4):
        A1s[h * 4 : h * 4 + 4, h] = a_src1[h]
        A1d[h * 4 : h * 4 + 4, h] = a_dst1[h]
    pack1 = np.concatenate([W1, W1 @ A1s, W1 @ A1d], axis=1)  # [512, 24]

    W2 = np.asarray(inputs["W2"], np.float32)
    a2s = np.asarray(inputs["a_src2"], np.float32)[:, 0]
    a2d = np.asarray(inputs["a_dst2"], np.float32)[:, 0]
    pack2 = np.concatenate([W2, W2 * a2s[None, :], W2 * a2d[None, :]], axis=1)  # [16,12]
    pk2rep = np.tile(pack2.T.reshape(1, 192), (128, 1)).astype(np.float32)

    b1rep = np.tile(np.asarray(inputs["b1"], np.float32)[None, :], (128, 1))
    b2rep = np.tile(np.asarray(inputs["b2"], np.float32)[None, :], (128, 1))
    wfcrep = np.tile(np.asarray(inputs["Wfc"], np.float32)[:, 0][None, :], (128, 1))
    bfc = float(np.asarray(inputs["bfc"])[0])

    xp = np.zeros((NPAD, F_IN), np.float32)
    vm = order < N_NODES
    xp[vm] = x[order[vm]]
    xpb = xp.reshape(784, 128, F_IN)

    mb = morder.reshape(784, 128)
    in_maps = []
    for c in range(8):
        xT_c = np.ascontiguousarray(
            xpb[c::8].reshape(SHARD, F_IN).T
        )
        smul_c = np.ascontiguousarray(mb[c::8].T)     # [128, NB]
        in_maps.append({
            "xT": xT_c, "pack1": pack1, "pk2rep": pk2rep,
            "b1rep": b1rep, "b2rep": b2rep, "wfcrep": wfcrep,
            "offs": np.ascontiguousarray(offs_all[c]),
            "smul": smul_c,
        })
    return cfg, in_maps, order, vm, bfc


def assemble_output(results, order, vm, bfc):
    ynew = np.zeros(NPAD, np.float32)
    yb = ynew.reshape(784, 128)
    for c in range(8):
        yb[c::8] = np.asarray(results[c]["yout"]).T  # [NB,128]
    y = np.empty(N_NODES, np.float32)
    y[order[vm]] = ynew[vm] + bfc
    return y[:, None]


LAST_EXEC_NS = None


def run(inputs: dict, trace: bool = False):
    cfg, in_maps, order, vm, bfc = host_prep(inputs)
    nc = build(cfg)
    res = run_bass_kernel_spmd(nc, in_maps, core_ids=list(range(8)), trace=trace)
    y = assemble_output(res.results, order, vm, bfc)
    return y, res


def kernel(**inputs) -> np.ndarray:
    global LAST_EXEC_NS
    trace = _install_axon_ntff_shim()
    try:
        y, res = run(inputs, trace=trace)
    except Exception:
        if not trace:
            raise
        y, res = run(inputs, trace=False)
    LAST_EXEC_NS = res.exec_time_ns
    return np.ascontiguousarray(y.astype(np.float32))
